# revision 75
# baseline (speedup 1.0000x reference)
"""Multi-head attention (B=2, T=2048, C=1024, H=16) on 8 TRN2 NeuronCores.

Sharding: core c = (b, g) with b = c // 4 (data parallel over batch),
g = c % 4 (tensor parallel over head groups of 4 heads = 256 cols).
Wq/Wk/Wv are column-sharded, Wp row-sharded (Megatron); the host sums the
4 partial output projections per batch and adds the bias.

Per-core pipeline (shapes hardcoded for this problem):
  - host passes x^T [C, T]; q/k inputs and weights are fp8 e4m3 (scaled by
    8 into the e4m3 normal range; 1/(8*8*32) is folded into the exp), v
    stays bf16 for accuracy
  - Q/K projections run as fp8 DoubleRow matmuls (two 128-deep k-tiles per
    instruction, half the PE rows), copied out f32r as Q^T/K^T [64*2, T]
    (partition = head dim, head pairs stacked); V is token-major
    [128, kc, h, 65] with a ones column appended for the softmax
    denominator
  - phase A covers K fully, then Q and V for the first 1024 tokens only;
    the second halves of Q and V are deferred items executed inside the
    first attention iterations (the PE has slack there), with their x
    DMAs deadline-ordered behind the phase A traffic on the SP queue
  - attention per (q-half, head): scores S^T[k, q] = K_h^T.T @ Q_h^T one
    128-row k-chunk at a time into [128, 1024] PSUM (f32r runs at full
    rate at this width); exp on ACT with scale folded in (no max
    subtraction; |S| is bounded at these scales), mask as a bf16 {0,1}
    multiply on DVE (2x mode); ACT is the pacing engine at ~1us per
    k-chunk
  - P^T @ V_aug accumulates O^T [65, 1024] (one PSUM accumulation group
    per bank); the PV matmuls trail the scores by four k-chunks so the
    in-order PE never stalls waiting for exp/mask
  - normalization: reciprocal of row 64, broadcast to 64 partitions with
    a K=1 matmul, then copy+multiply into a head-pair-stacked ot2
    [128, T] bf16 layout (partition-shifted copy for odd heads); these
    "finish" steps are deferred one iteration and popped inside the next
    k-loop so their latency hides
  - output projection contracts 128 rows per head pair (2 matmuls per
    512-col tile); tiles for the first q-half interleave into the second
    q-half's k-loops (one per 3 k-chunks), the rest drain in a short
    tail with PSUM-pool and copy-engine alternation
  - y is written bf16; the host sums the 4 partials in f32 and adds bias
"""
import numpy as np
import ml_dtypes

import bass_rust
import concourse.bass as bass
import concourse.mybir as mybir
import concourse.tile as tile
from concourse.bass_utils import run_bass_kernel_spmd
from concourse.vector_clock import ScopedClock

# ---------------------------------------------------------------------------
# Workaround: walrus rejects >~4 sync waits on one instruction; the Tile exit
# drain aggregates one wait per DMA queue/engine.  Spread them over a chain of
# single-wait NOPs on the sync engine before draining.
# ---------------------------------------------------------------------------


def _patched_drain_and_barrier(self, tick_clock, wait_clock):
    nc = self.nc
    probe = nc.sync.nop(nofuse=True)
    wait_clock.add_sem_waits(probe.ins, ScopedClock({None: tick_clock.global_clock}))
    waits = list(probe.ins.sync_info.on_wait) if probe.ins.sync_info else []
    probe.ins.sync_info = bass_rust.SyncInfo(
        on_wait=waits[:1], on_update=[]
    )
    for w in waits[1:]:
        n = nc.sync.nop(nofuse=True)
        n.ins.sync_info = bass_rust.SyncInfo(on_wait=[w], on_update=[])

    nc.sync.drain()
    nc.all_engine_barrier()
    assert self.sems is not None
    popped = nc._tile_sem_poison_stack.pop()
    assert popped is self._sem_poison
    nc.clear_and_free_semaphores(list(self.sems.allocated().values()))
    nc.all_engine_barrier()


tile.TileContext._drain_and_barrier = _patched_drain_and_barrier

_MAX_WAITS = 1


def _split_excess_waits(nc, limit=_MAX_WAITS):
    """Walrus codegen allows only ONE sync wait on compute instructions
    (more on CTRL, but be uniform).  For any instruction carrying more,
    peel the excess onto same-engine single-wait NOPs inserted immediately
    before it in the basic block."""
    n_new = 0
    for f in nc.m.functions:
        for bb in f.blocks:
            insts = bb.instructions
            out = []
            for inst in insts:
                si = inst.sync_info
                waits = list(si.on_wait) if si and si.on_wait else []
                if len(waits) > limit:
                    extra, keep = waits[:-limit], waits[-limit:]
                    inst.sync_info = bass_rust.SyncInfo(
                        on_wait=keep, on_update=list(si.on_update)
                    )
                    for j in range(0, len(extra), limit):
                        nop = mybir.InstNoOp(
                            name=f"waitsplit-{n_new}",
                            engine=inst.engine,
                            ins=[],
                            outs=[],
                            sync_info=bass_rust.SyncInfo(
                                on_wait=extra[j:j + limit], on_update=[]
                            ),
                        )
                        n_new += 1
                        out.append(nop)
                out.append(inst)
            if n_new:
                bb.instructions = out
    return n_new

# ---------------------------------------------------------------------------

B, T, C, H = 2, 2048, 1024, 16
GROUPS = 4                 # head groups (tensor parallel width per batch)
HG = H // GROUPS           # 4 heads per group
DH = C // H                # 64
COLS = HG * DH             # 256 local columns
KC = T // 128              # 16 k-chunks of 128
CC = C // 128              # 8 contraction chunks for the projections
QCB = T // 512             # 4 token chunks of 512 in phase A

F32 = mybir.dt.float32
F32R = mybir.dt.float32r
BF16 = mybir.dt.bfloat16


def _mm(nc, out, lhsT, rhs, start, stop):
    nc.tensor.matmul(out, lhsT, rhs, start=start, stop=stop)


def build_program(split_waits=True):
    nc = bass.Bass("TRN2", target_bir_lowering=False, debug=False, num_devices=8)

    FP8 = mybir.dt.float8e4
    xqT = nc.declare_dram_parameter("xqT", [C, T], FP8, isOutput=False)
    xkT = nc.declare_dram_parameter("xkT", [C, T], FP8, isOutput=False)
    xvT = nc.declare_dram_parameter("xvT", [C, T], BF16, isOutput=False)
    maskT = nc.declare_dram_parameter("maskT", [T, T], BF16, isOutput=False)
    wq = nc.declare_dram_parameter("wq", [C, COLS], FP8, isOutput=False)
    wk = nc.declare_dram_parameter("wk", [C, COLS], FP8, isOutput=False)
    wv = nc.declare_dram_parameter("wv", [C, COLS], BF16, isOutput=False)
    wp = nc.declare_dram_parameter("wp", [COLS, C], BF16, isOutput=False)
    y = nc.declare_dram_parameter("y", [T, C], BF16, isOutput=True)

    with tile.TileContext(nc) as tc:
        import contextlib
        with contextlib.ExitStack() as ctx:
            persist = ctx.enter_context(tc.tile_pool(name="persist", bufs=1))

            # persistent SBUF tensors
            mask_sb = persist.tile([128, KC, T], BF16)        # 64 KB/part
            qt_sb = persist.tile([128, 2, T], F32R)           # 16 KB/part
            kt_sb = persist.tile([128, 2, T], F32R)           # 16 KB/part
            vaug_sb = persist.tile([128, KC, HG, DH + 1], BF16)  # 8.1 KB/part
            ot2_sb = [
                persist.tile([128, T], BF16, tag=f"ot{p}", name=f"ot2_sb{p}")
                for p in range(2)
            ]
            wp_sb = persist.tile([128, 2, C], BF16)           # 4 KB/part
            ones_f32 = persist.tile([1, DH], F32)

            nc.vector.memset(vaug_sb[:, :, :, DH:DH + 1], 1.0)
            nc.vector.memset(ones_f32, 1.0)
            ones_sb = ones_f32.bitcast(F32R)

            # ---------------- Phase A: projections ---------------------------
            # K fully; V and Q only for the first 1024 tokens.  The second
            # halves of V and Q are deferred into the first q-half's
            # attention iterations (the fp8 score matmuls leave PE slack).
            pw = ctx.enter_context(tc.tile_pool(name="pa_w", bufs=1))
            wq_sb = pw.tile([128, CC, COLS], FP8)
            wk_sb = pw.tile([128, CC, COLS], FP8)
            wv_sb = pw.tile([128, CC, COLS], BF16)
            nc.gpsimd.dma_start(wq_sb, wq.rearrange("(cc p) n -> p cc n", p=128))
            nc.gpsimd.dma_start(wk_sb, wk.rearrange("(cc p) n -> p cc n", p=128))
            nc.gpsimd.dma_start(wv_sb, wv.rearrange("(cc p) n -> p cc n", p=128))
            nc.gpsimd.dma_start(wp_sb, wp.rearrange("(g p) n -> p g n", p=128))

            # x tiles for the deferred second halves (alive through phase B)
            pdef = ctx.enter_context(tc.tile_pool(name="pa_def", bufs=1))
            xv_d = [pdef.tile([128, CC, 512], BF16, tag=f"xvd{i}",
                              name=f"xv_d{i}") for i in range(2)]
            xq_d = [pdef.tile([128, CC, 512], FP8, tag=f"xqd{i}",
                              name=f"xq_d{i}") for i in range(2)]
            # K/Q projections: fp8 DoubleRow, two 128-deep k-tiles per
            # matmul (contraction pairs along the cc axis); the result is
            # copied out f32r in the [col, q] head-dim-major layout the
            # score matmuls consume directly.
            def qk_proj_chunk(x_dram, w_sb, out_sb, px, pp, xtag, ptag,
                              qc, dma=True, x_t=None):
                qs = slice(qc * 512, (qc + 1) * 512)
                if x_t is None:
                    x_t = px.tile([128, CC, 512], FP8, tag=xtag,
                                  name=f"{xtag}{qc}")
                if dma:
                    src = x_dram[:, qs].rearrange("(cc p) q -> p cc q", p=128)
                    nc.sync.dma_start(x_t[:, 0:4], src[:, 0:4])
                    nc.sync.dma_start(x_t[:, 4:8], src[:, 4:8])
                o_ps = pp.tile([128, 2, 512], F32, tag="aps",
                               name=f"{ptag}{qc}")
                for c2 in range(CC // 2):
                    st, sp = c2 == 0, c2 == CC // 2 - 1
                    cs = slice(2 * c2, 2 * c2 + 2)
                    for mh in range(2):
                        m = slice(mh * 128, (mh + 1) * 128)
                        nc.tensor.matmul(
                            o_ps[:, mh], w_sb[:, cs, m], x_t[:, cs],
                            start=st, stop=sp,
                            perf_mode=mybir.MatmulPerfMode.DoubleRow)
                for mh in range(2):
                    nc.scalar.copy(out_sb[:, mh, qs], o_ps[:, mh])

            def v_proj_tt(v_ps, xv_t, qc, tt):
                for cc in range(CC):
                    _mm(nc, v_ps[:, 0:COLS],
                        xv_t[:, cc, tt * 128:(tt + 1) * 128],
                        wv_sb[:, cc], cc == 0, cc == CC - 1)
                nc.vector.tensor_copy(
                    vaug_sb[:, qc * 4 + tt, :, 0:DH],
                    v_ps[:, 0:COLS].rearrange("p (h d) -> p h d", h=HG))

            def mask_dma(qh, kcs):
                qsl = slice(qh * 1024, (qh + 1) * 1024)
                for kc in kcs:
                    nc.sync.dma_start(
                        mask_sb[:, kc, qsl],
                        maskT[kc * 128:(kc + 1) * 128, qsl])

            # one shared pool set for the whole phase so sections overlap
            # (sequential pools would serialize on memory-reuse waits)
            with tc.tile_pool(name="pa_x", bufs=2) as pa_x, \
                 tc.tile_pool(name="pa_ps", bufs=2, space="PSUM") as pap:
                # K all four chunks; x DMAs for K/Q/V emitted first on SP
                # Q01 first (shortest path to the first scores),
                # then K, then V01
                for qc in range(2):
                    qk_proj_chunk(xqT, wq_sb, qt_sb, pa_x, pap,
                                  "xq", "qtps", qc)
                def v_chunk(qc):
                    qs = slice(qc * 512, (qc + 1) * 512)
                    xv_t = pa_x.tile([128, CC, 512], BF16, tag="xv",
                                     name=f"xv_t{qc}")
                    src = xvT[:, qs].rearrange("(cc p) q -> p cc q", p=128)
                    nc.sync.dma_start(xv_t[:, 0:4], src[:, 0:4])
                    nc.sync.dma_start(xv_t[:, 4:8], src[:, 4:8])
                    v_ps = pap.tile([128, 4, 512], F32, tag="aps",
                                    name=f"vps{qc}")
                    for cc in range(CC):
                        st, sp = cc == 0, cc == CC - 1
                        for tt in range(4):
                            _mm(nc, v_ps[:, tt, 0:COLS],
                                xv_t[:, cc, tt * 128:(tt + 1) * 128],
                                wv_sb[:, cc], st, sp)
                    for tt in range(4):
                        nc.vector.tensor_copy(
                            vaug_sb[:, qc * 4 + tt, :, 0:DH],
                            v_ps[:, tt, 0:COLS].rearrange(
                                "p (h d) -> p h d", h=HG),
                        )

                # V chunks interleave between K chunks to fill the PE gaps
                # the xk tile-rotation DMA waits would otherwise leave
                for qc in range(2):
                    qk_proj_chunk(xkT, wk_sb, kt_sb, pa_x, pap,
                                  "xk", "ktps", qc)
                v_chunk(0)
                for qc in range(2, QCB):
                    qk_proj_chunk(xkT, wk_sb, kt_sb, pa_x, pap,
                                  "xk", "ktps", qc)
                v_chunk(1)
                # deferred-x DMAs + masks, ordered by consumption deadline
                mask_dma(0, range(0, 4))
                for i, x_t in enumerate(xv_d):
                    src = xvT[:, (2 + i) * 512:(3 + i) * 512].rearrange(
                        "(cc p) q -> p cc q", p=128)
                    nc.sync.dma_start(x_t[:, 0:4], src[:, 0:4])
                    nc.sync.dma_start(x_t[:, 4:8], src[:, 4:8])
                mask_dma(0, range(4, 16))
                for i, x_t in enumerate(xq_d):
                    src = xqT[:, (2 + i) * 512:(3 + i) * 512].rearrange(
                        "(cc p) q -> p cc q", p=128)
                    nc.sync.dma_start(x_t[:, 0:4], src[:, 0:4])
                    nc.sync.dma_start(x_t[:, 4:8], src[:, 4:8])
                mask_dma(1, range(0, 16))

            # ---------------- Phase B + C interleaved ----------------------
            with tc.tile_pool(name="ps_all", bufs=2, space="PSUM") as pps, \
                 tc.tile_pool(name="ps_ot", bufs=2, space="PSUM") as ppo, \
                 tc.tile_pool(name="pt", bufs=6) as ppt, \
                 tc.tile_pool(name="rc", bufs=2) as prc, \
                 tc.tile_pool(name="ysb", bufs=4) as pyt:

                y_tiles = {}

                def make_tt_half(tt, nk, copy_eng=None, pool=None):
                    def emit():
                        trange = slice(tt * 128, (tt + 1) * 128)
                        ns = slice(nk * 512, (nk + 1) * 512)
                        if nk == 0:
                            y_tiles[tt] = pyt.tile([128, C], BF16, tag="y",
                                                   name=f"y_t{tt}")
                        p = pool or pps
                        tag = "s" if p is pps else "ot"
                        y_ps = p.tile([128, 512], F32, tag=tag,
                                      name=f"y_ps{tt}_{nk}")
                        for hp in range(2):
                            _mm(nc, y_ps, ot2_sb[hp][:, trange],
                                wp_sb[:, hp, ns], hp == 0, hp == 1)
                        eng = copy_eng or nc.vector
                        if eng is nc.scalar:
                            eng.copy(y_tiles[tt][:, ns], y_ps)
                        else:
                            eng.tensor_copy(y_tiles[tt][:, ns], y_ps)
                        if nk == 1:
                            nc.sync.dma_start(y[trange, :], y_tiles[tt])
                    return emit

                def make_finish(hp, hh, qh, ot_ps, rc_t):
                    def emit():
                        # broadcast 1/denominator over 64 partitions via a
                        # K=1 matmul, then evacuate O^T normalized into the
                        # head-pair-stacked ot2 layout
                        pb = 64 * hh
                        qsl = slice(qh * 1024, (qh + 1) * 1024)
                        bc_ps = pps.tile([DH, 1024], F32, tag="s",
                                         name=f"bc{qh}{hp}{hh}")
                        for j in range(2):
                            _mm(nc, bc_ps[:, j * 512:(j + 1) * 512], ones_sb,
                                rc_t[:, j * 512:(j + 1) * 512], True, True)
                        dst = ot2_sb[hp][pb:pb + 64, qsl]
                        nc.vector.tensor_copy(dst, ot_ps[0:DH, :])
                        nc.vector.tensor_mul(dst, dst, bc_ps)
                    return emit

                urgent = []   # normalization/evacuation: pop 1 per k-chunk
                lazy = []     # projection tiles: pop 1 per 3 k-chunks

                # deferred V (tokens 1024-2047) and Q (q-half 1) projections,
                # run inside the first attention iterations
                def make_v_item(i, tt):
                    def emit():
                        v_ps = pps.tile([128, 512], F32, tag="s",
                                        name=f"vd{i}{tt}")
                        v_proj_tt(v_ps, xv_d[i], 2 + i, tt)
                    return emit

                def make_q_item(i):
                    def emit():
                        qc = 2 + i
                        qs = slice(qc * 512, (qc + 1) * 512)
                        o_ps = pps.tile([128, 2, 512], F32, tag="s",
                                        name=f"qd{i}")
                        for c2 in range(CC // 2):
                            st, sp = c2 == 0, c2 == CC // 2 - 1
                            cs = slice(2 * c2, 2 * c2 + 2)
                            for mh in range(2):
                                m = slice(mh * 128, (mh + 1) * 128)
                                nc.tensor.matmul(
                                    o_ps[:, mh], wq_sb[:, cs, m],
                                    xq_d[i][:, cs], start=st, stop=sp,
                                    perf_mode=mybir.MatmulPerfMode.DoubleRow)
                        for mh in range(2):
                            nc.vector.tensor_copy(qt_sb[:, mh, qs],
                                                  o_ps[:, mh])
                    return emit

                for i in range(2):
                    for tt in range(4):
                        urgent.append(make_v_item(i, tt))
                urgent.append(make_q_item(0))
                urgent.append(make_q_item(1))

                for qh in range(2):
                    qsl = slice(qh * 1024, (qh + 1) * 1024)
                    if qh == 1:
                        # output projection for the finished first q-half
                        for tt in range(8):
                            for nk in range(2):
                                lazy.append(make_tt_half(tt, nk))
                    for hp in range(2):
                        for hh in range(2):
                            h = 2 * hp + hh
                            pb = 64 * hh
                            kt_h = kt_sb[pb:pb + 64, hp]
                            qt_h = qt_sb[pb:pb + 64, hp]
                            ot_ps = ppo.tile([DH + 1, 1024], F32, tag="ot",
                                             name=f"ot{qh}{hp}{hh}")
                            pts = [None] * KC

                            def emit_pv(kc):
                                for j in range(2):
                                    _mm(nc, ot_ps[:, j * 512:(j + 1) * 512],
                                        vaug_sb[:, kc, h],
                                        pts[kc][:, j * 512:(j + 1) * 512],
                                        kc == 0, kc == KC - 1)

                            for kc in range(KC):
                                s_ps = pps.tile([128, 1024], F32, tag="s",
                                                name=f"s{kc}")
                                ks = slice(kc * 128, (kc + 1) * 128)
                                for j in range(2):
                                    qq = slice(qh * 1024 + j * 512,
                                               qh * 1024 + (j + 1) * 512)
                                    _mm(nc, s_ps[:, j * 512:(j + 1) * 512],
                                        kt_h[:, ks], qt_h[:, qq], True, True)
                                if kc >= 4:
                                    emit_pv(kc - 4)
                                if kc >= 2:
                                    if urgent:
                                        urgent.pop(0)()
                                    elif lazy and kc % 3 == 2:
                                        lazy.pop(0)()
                                pt_t = ppt.tile([128, 1024], BF16, tag="pt",
                                                name=f"pt{kc}")
                                nc.scalar.activation(
                                    pt_t, s_ps,
                                    mybir.ActivationFunctionType.Exp,
                                    scale=1.0 / 2048.0,
                                )
                                nc.vector.tensor_mul(
                                    pt_t, pt_t, mask_sb[:, kc, qsl])
                                pts[kc] = pt_t

                            emit_pv(KC - 4)
                            emit_pv(KC - 3)
                            emit_pv(KC - 2)
                            emit_pv(KC - 1)
                            rc_t = prc.tile([1, 1024], F32R, tag="rc",
                                            name=f"rc{qh}{hp}{hh}")
                            with nc.allow_low_precision(reason="softmax recip"):
                                nc.vector.reciprocal(rc_t, ot_ps[DH:DH + 1, :])
                            urgent.append(
                                make_finish(hp, hh, qh, ot_ps, rc_t))

                # drain deferred normalizations, then the projection tail
                for emit in urgent + lazy:
                    emit()
                for tt in range(8, 16):
                    for nk in range(2):
                        eng = nc.scalar if (tt + nk) % 2 == 0 else nc.vector
                        pool = ppo if tt % 2 == 0 else pps
                        make_tt_half(tt, nk, copy_eng=eng, pool=pool)()

    if split_waits:
        _split_excess_waits(nc)
    return nc


_program_cache = None


def _get_program():
    global _program_cache
    if _program_cache is None:
        _program_cache = build_program()
    return _program_cache


def make_in_maps(query, key, value, mask, Wq, Wk, Wv, Wp):
    # fp8 path: q/k weights and activations are scaled by 8 to sit in the
    # e4m3 normal range; the kernel folds 1/(8*8*32) = 1/2048 into the exp
    # (32 = C**0.5 is the reference's score scale)
    fp8 = ml_dtypes.float8_e4m3

    in_maps = []
    for c in range(8):
        b, g = c // GROUPS, c % GROUPS
        cols = slice(g * COLS, (g + 1) * COLS)
        in_maps.append({
            "xqT": np.ascontiguousarray(query[b].T).astype(fp8),
            "xkT": np.ascontiguousarray(key[b].T).astype(fp8),
            "xvT": np.ascontiguousarray(value[b].T).astype(ml_dtypes.bfloat16),
            "maskT": np.ascontiguousarray(mask[b].T).astype(ml_dtypes.bfloat16),
            "wq": np.ascontiguousarray(Wq[:, cols] * 8).astype(fp8),
            "wk": np.ascontiguousarray(Wk[:, cols] * 8).astype(fp8),
            "wv": np.ascontiguousarray(Wv[:, cols]).astype(ml_dtypes.bfloat16),
            "wp": np.ascontiguousarray(Wp[cols, :]).astype(ml_dtypes.bfloat16),
        })
    return in_maps


def assemble_output(res, bp):
    out = np.empty((B, T, C), np.float32)
    for b in range(B):
        acc = res.results[b * GROUPS]["y"].astype(np.float32)
        for g in range(1, GROUPS):
            acc = acc + res.results[b * GROUPS + g]["y"].astype(np.float32)
        out[b] = acc + bp
    return out


def kernel(query, key, value, mask, Wq, Wk, Wv, Wp, bp):
    query = np.asarray(query, np.float32)
    key = np.asarray(key, np.float32)
    value = np.asarray(value, np.float32)
    mask = np.asarray(mask)
    Wq = np.asarray(Wq, np.float32)
    Wk = np.asarray(Wk, np.float32)
    Wv = np.asarray(Wv, np.float32)
    Wp = np.asarray(Wp, np.float32)
    bp = np.asarray(bp, np.float32)

    in_maps = make_in_maps(query, key, value, mask, Wq, Wk, Wv, Wp)
    nc = _get_program()
    res = run_bass_kernel_spmd(nc, in_maps, list(range(8)))
    return assemble_output(res, bp)
